# revision 33
# baseline (speedup 1.0000x reference)
"""Contrastive loss kernel for Trainium2 (8 NeuronCores, Bass/Tile).

Strategy
--------
Only rows with label==1 (pos) contribute losses, and only columns with
label==0 (neg) plus the diagonal enter each row's logsumexp.  The host
computes the tiny index sets from `labels`, then each of the 8 cores
(2 per batch) receives:
  gp : its half of the batch's positive greek rows, row-major [P1,256]
  gpt: the same rows pre-transposed on host [2,128,P1] (H on partitions)
  ep : english rows at the same indices (diag term)    [P1,256]
  en : all negative english rows of the batch          [N1,256]
all bf16, zero-padded to uniform compile-time shapes (P1, N1).  N2 <= N1
is the exact used width of the neg axis (matmul/exp only touch N2
columns; transposes run on full 128-row chunks).

Device pipeline: row sums-of-squares via fused square+accumulate ops
(DVE scalar_tensor_tensor + Scalar-engine Square activations, which
share the Ln/Exp table so no reload), inverse norms via per-piece
Ln / Exp(scale=-0.5) pairs.  The greek scale (1/T folded via T^2 in its
squares) is applied as the per-partition `scale` operand of the exp
pass, so raw host-transposed gpt feeds the matmul directly.  English
rows are scaled with 4x-mode tensor_scalar ops, PE-transposed, and
copied to SBUF.  Logits accumulate in PSUM (bf16 matmul); one in-place
exp(s_g*x - 15) pass with accumulate per 128-row pos chunk produces the
negative sums.  Zero-padded en columns give exactly 0 logits; their
exp(-15) mass is removed by an exact host-computed correction.  The
diag path (second gp piece, ep norms, dots) runs in the exp-phase slack
via scheduler wait hints, with its two rsqrt pairs slotted between
exps.  The device ships ln(S+corr+e^(d-15)) and d per row; the host
applies the wv mask: loss = sum wv*(ln(...) + 15 - d) / count.
"""

import sys

if "/opt/trn_rl_repo" not in sys.path:
    sys.path.insert(0, "/opt/trn_rl_repo")

from contextlib import ExitStack

import ml_dtypes
import numpy as np

import concourse.bass as bass
import concourse.tile as tile
from concourse import mybir
from concourse.bass_utils import run_bass_kernel_spmd
from concourse.masks import make_identity

TEMPERATURE = 0.07
IGNORE_INDEX = -100
CMAX = 15.0
H = 256
N_CORES = 8

# Stash of the most recent BassKernelResults + shapes (for test harness timing).
LAST_RESULTS = None
LAST_SHAPES = None
TRACE = False


def _legalize_waits(nc: bass.Bass, max_waits: int = 1) -> None:
    """This container's walrus accepts at most one sync-wait per instruction
    (ACT structs especially); Tile can emit several.  Split the excess onto
    same-engine NoOps placed immediately before the instruction."""
    for bb in nc.main_func.blocks:
        new = []
        for ins in bb.instructions:
            si = ins.sync_info
            if si is not None and si.on_wait and len(si.on_wait) > max_waits:
                waits = list(si.on_wait)
                extra, keep = waits[:-max_waits], waits[-max_waits:]
                for i in range(0, len(extra), max_waits):
                    new.append(
                        mybir.InstNoOp(
                            name=nc.get_next_instruction_name(),
                            engine=ins.engine,
                            ins=[],
                            outs=[],
                            sync_info=mybir.SyncInfo(
                                on_wait=extra[i : i + max_waits], on_update=[]
                            ),
                            bass_nofuse=True,
                        )
                    )
                ins.sync_info = mybir.SyncInfo(
                    on_wait=keep, on_update=list(si.on_update or [])
                )
            new.append(ins)
        bb.instructions[:] = new


def _build_program(P1: int, N1: int, N2: int, legalize: bool = True) -> bass.Bass:
    """One SPMD program: shapes P1 (pos rows) / N1 (padded neg rows) / N2
    (used neg width) are uniform across cores; data differs via in_maps."""
    PC = P1 // 128
    NC = N1 // 128
    GA = min(2, PC)  # early greek piece (chunks [0:GA])
    GB = PC - GA
    assert N2 <= N1 <= 1536 and P1 <= 1664
    f32 = mybir.dt.float32
    bf16 = mybir.dt.bfloat16
    OP = mybir.AluOpType
    AF = mybir.ActivationFunctionType

    # en DMA pieces: first two on SP, rest on Pool SWDGE
    bounds = sorted({0, min(3, NC), min(6, NC), min(9, NC), NC})
    pieces = [(bounds[i], bounds[i + 1]) for i in range(len(bounds) - 1)]
    piece_q = [0, 0, 1, 1][: len(pieces)]
    # ACT Square handles these en chunks (plus greek piece A)
    act_sq_en = {0, 1, 3} if NC > 5 else set()
    # 512-wide matmul tiles over the used width N2
    nts = []
    c0 = 0
    while c0 < N2:
        w = min(512, N2 - c0)
        nts.append((c0, w))
        c0 += w

    nc = bass.Bass()
    gp = nc.dram_tensor("gp", [P1, H], bf16, kind="ExternalInput")
    gpt = nc.dram_tensor("gpt", [2, 128, P1], bf16, kind="ExternalInput")
    ep = nc.dram_tensor("ep", [P1, H], bf16, kind="ExternalInput")
    en = nc.dram_tensor("en", [N1, H], bf16, kind="ExternalInput")
    corr = nc.dram_tensor("corr", [1, 1], f32, kind="ExternalInput")
    out = nc.dram_tensor("out", [128, 2 * PC], f32, kind="ExternalOutput")

    with tile.TileContext(nc) as tc, ExitStack() as ctx:
        persist = ctx.enter_context(tc.tile_pool(name="persist", bufs=1))
        small = ctx.enter_context(tc.tile_pool(name="small", bufs=1))
        scratch = ctx.enter_context(tc.tile_pool(name="scratch", bufs=4))
        psum_tp = ctx.enter_context(tc.tile_pool(name="psum_tp", bufs=2, space="PSUM"))
        psum_mm = ctx.enter_context(tc.tile_pool(name="psum_mm", bufs=2, space="PSUM"))

        # ---- constants (DVE; the Pool queue is hogged by SWDGE gens) and a
        # tiny PE warmup matmul that anchors the p-state ramp at ~0.3us so
        # real matmuls (~3.5us in) run at full clock.
        with tc.high_priority():
            cneg_t = small.tile([128, 1], f32)
            nc.vector.memset(cneg_t[:], -CMAX)
            eps_t = small.tile([128, 1], f32)
            nc.vector.memset(eps_t[:], 1e-24)

        # ---- identity: DVE memset + one high-priority Pool affine_select
        # (the rest of the Pool queue is SWDGE descriptor generation).
        with tc.high_priority():
            ident = small.tile([128, 128], bf16)
            nc.vector.memset(ident[:], 0.0)
            make_identity(nc, ident[:], nomemset=True)

        # ---- ACT queue: table preload first, then nothing until squares.
        dummy = small.tile([128, 1], f32)
        nc.scalar.activation(
            out=dummy[:], in_=eps_t[:], func=AF.Ln, bias=eps_t[:, 0:1], scale=1.0
        )

        # PE warmup + keep-warm dummies: anchor and hold the p-state ramp so
        # the real transposes/matmuls (~5us+) run at/near full clock.
        with tc.high_priority():
            ptw = psum_tp.tile([128, 768], bf16, tag="pt")
            for _ in range(30):
                nc.tensor.transpose(ptw[:, 0:128], ident[:], ident[:])

        GfA = persist.tile([128, GA, H], bf16, name="gfa")
        gp_r = gp[:].rearrange("(c p) h -> p c h", p=128)
        nc.sync.dma_start(out=GfA[:], in_=gp_r[:, 0:GA, :])

        en_r = en[:].rearrange("(c p) h -> p c h", p=128)
        EnP = []
        for qi, (lo, hi) in zip(piece_q, pieces):
            t = persist.tile([128, hi - lo, H], bf16, tag=f"en{lo}", name=f"en{lo}")
            eng = [nc.sync, nc.gpsimd][qi]
            eng.dma_start(out=t[:], in_=en_r[:, lo:hi, :])
            EnP.append(t)

        def en_chunk(c):
            for (lo, hi), t in zip(pieces, EnP):
                if lo <= c < hi:
                    return t[:, c - lo, :]
            raise AssertionError

        GfB = None
        if GB:
            GfB = persist.tile([128, GB, H], bf16, name="gfb")
            nc.sync.dma_start(out=GfB[:], in_=gp_r[:, GA:PC, :])
        corr_t = small.tile([128, 1], f32)
        nc.sync.dma_start(out=corr_t[:], in_=corr[:].to_broadcast([128, 1]))

        GT = persist.tile([128, 2, P1], bf16)
        nc.gpsimd.dma_start(out=GT[:], in_=gpt[:].rearrange("k p j -> p k j"))
        # Ef's SWDGE gen is emitted later (after the applies) so it doesn't
        # block them in the Pool queue; tile declared here for use below.
        Ef = persist.tile([128, PC, H], bf16)

        def gf_chunk(c):
            if c < GA:
                return GfA[:, c, :]
            return GfB[:, c - GA, :]

        # ---- row sums of squares ----------------------------------------
        ss = small.tile([128, NC + GA], f32)

        def sq_dve(src, ss_t, col, scalar=1.0, tag="sqd"):
            sq = scratch.tile([128, H], bf16, tag=tag)
            nc.vector.scalar_tensor_tensor(
                out=sq[:], in0=src, scalar=scalar, in1=src,
                op0=OP.mult, op1=OP.mult,
                accum_out=ss_t[:, col : col + 1],
            )

        def sq_act(src, ss_t, col, scale=1.0):
            sq = scratch.tile([128, H], bf16, tag="sqa")
            nc.scalar.activation(
                out=sq[:], in_=src, func=AF.Square, bias=0.0, scale=scale,
                accum_out=ss_t[:, col : col + 1],
            )

        # ACT: en {3,4,5} + greek piece A ; DVE: the rest
        for c in act_sq_en:
            sq_act(en_chunk(c), ss, c)
        for c in range(GA):
            sq_act(GfA[:, c, :], ss, NC + c, scale=float(TEMPERATURE))
        dve_order = [c for c in (6, 7, 8, 2, 4, 5, 9, 10, 11) if c < NC and c not in act_sq_en]
        dve_order += [c for c in range(NC) if c not in act_sq_en and c not in dve_order]
        for c in dve_order:
            sq_dve(en_chunk(c), ss, c)

        # ---- inverse norms: s = (ss+eps)^-0.5 via Ln + Exp, in halves
        s = small.tile([128, NC + GA], f32)

        def rsqrt(dst, src, lo, hi):
            nc.scalar.activation(
                out=dst[:, lo:hi], in_=src[:, lo:hi], func=AF.Ln,
                bias=eps_t[:, 0:1], scale=1.0,
            )
            nc.scalar.activation(
                out=dst[:, lo:hi], in_=dst[:, lo:hi], func=AF.Exp,
                bias=0.0, scale=-0.5,
            )

        rsqrt(s, ss, NC, NC + GA)  # greek piece A (exp scales chunks < GA)
        half = min(6, NC)
        rsqrt(s, ss, 0, half)
        if NC > half:
            rsqrt(s, ss, half, NC)

        # ---- scale en rows (4x-mode DVE / Pool split), transpose ---------
        # transposed chunks collect into 768-wide PSUM tiles -> fewer copies
        Enb = persist.tile([128, NC, H], bf16)
        NbT = persist.tile([128, 2, N1], bf16)
        for pi, (lo, hi) in enumerate(pieces):
            for c in range(lo, hi):
                eng = nc.vector if c % 2 == 0 else nc.gpsimd
                eng.tensor_scalar_mul(Enb[:, c, :], en_chunk(c), s[:, c : c + 1])
        for g0 in range(0, NC, 6):
            gn = min(6, NC - g0)
            for hk in range(2):
                pt = psum_tp.tile([128, 768], bf16, tag="pt")
                for j in range(gn):
                    nc.tensor.transpose(
                        pt[:, j * 128 : (j + 1) * 128],
                        Enb[:, g0 + j, hk * 128 : (hk + 1) * 128],
                        ident[:],
                    )
                if hk == 1:
                    nc.scalar.copy(
                        out=NbT[:, hk, g0 * 128 : (g0 + gn) * 128],
                        in_=pt[:, : gn * 128],
                    )
                else:
                    nc.vector.tensor_copy(
                        out=NbT[:, hk, g0 * 128 : (g0 + gn) * 128],
                        in_=pt[:, : gn * 128],
                    )

        # Ef load: emitted after the applies so its Pool SWDGE descriptor
        # generation is ordered behind them in the Pool queue.
        nc.gpsimd.dma_start(out=Ef[:], in_=ep[:].rearrange("(c p) h -> p c h", p=128))

        # ---- diag path: wait hints push it into the exp-phase slack ------
        ssl = small.tile([128, GB + PC], f32)  # [g_late | ep]
        dot = small.tile([128, PC], f32)
        with tc.tile_wait_until(0.0085):
            for c in range(GA, PC):
                sq_dve(gf_chunk(c), ssl, c - GA, scalar=float(TEMPERATURE**2), tag="sqe")
        with tc.tile_wait_until(0.0105):
            for c in range(PC):
                sq_dve(Ef[:, c, :], ssl, GB + c, tag="sqe")
        with tc.tile_wait_until(0.0125):
            for c in range(PC):
                dsq = scratch.tile([128, H], bf16, tag="sqe")
                nc.vector.scalar_tensor_tensor(
                    out=dsq[:], in0=gf_chunk(c), scalar=1.0, in1=Ef[:, c, :],
                    op0=OP.mult, op1=OP.mult,
                    accum_out=dot[:, c : c + 1],
                )
        sl = small.tile([128, GB + PC], f32)
        se = sl[:, GB:]
        ltd = small.tile([128, 2 * PC], f32)
        diagn = ltd[:, PC:]
        ed = small.tile([128, PC], f32)
        t2 = small.tile([128, PC], f32)

        def emit_diag_tail():
            sg = small.tile([128, PC], f32)
            nc.vector.tensor_copy(out=sg[:, 0:GA], in_=s[:, NC : NC + GA])
            if GB:
                nc.vector.tensor_copy(out=sg[:, GA:], in_=sl[:, 0:GB])
            nc.vector.tensor_tensor(out=diagn, in0=dot[:], in1=sg[:], op=OP.mult)
            nc.vector.tensor_tensor(out=diagn, in0=diagn, in1=se, op=OP.mult)

        # ---- logits + one fused in-place exp/accumulate pass per chunk --
        # S[p, c] = sum_{q<N2} exp(s_g[p,c] * logit[c*128+p, q] - CMAX)
        S = small.tile([128, PC], f32)
        for c in range(PC):
            pm = psum_mm.tile([128, N2], f32, tag="pm")
            for t0, w in nts:
                for hk in range(2):
                    nc.tensor.matmul(
                        pm[:, t0 : t0 + w],
                        GT[:, hk, c * 128 : (c + 1) * 128],
                        NbT[:, hk, t0 : t0 + w],
                        start=(hk == 0),
                        stop=(hk == 1),
                    )
            if GB and c == min(2, PC - 1):
                rsqrt(sl, ssl, 0, GB)  # late greek exp scales
            if c == min(4, PC - 1):
                rsqrt(sl, ssl, GB, GB + PC)  # ep norms
                emit_diag_tail()
            if c == PC - 1:
                nc.scalar.activation(
                    out=ed[:], in_=diagn, func=AF.Exp,
                    bias=cneg_t[:, 0:1], scale=1.0,
                )
            scale_ap = s[:, NC + c : NC + c + 1] if c < GA else sl[:, c - GA : c - GA + 1]
            nc.scalar.activation(
                out=pm[:],
                in_=pm[:],
                func=AF.Exp,
                bias=cneg_t[:, 0:1],
                scale=scale_ap,
                accum_out=S[:, c : c + 1],
            )

        # t2 = S + corr + ed; chunks 0..PC-2 overlap the last exp, the
        # final chunk lands right after its accumulate.
        if PC > 1:
            nc.vector.scalar_tensor_tensor(
                out=t2[:, 0 : PC - 1], in0=S[:, 0 : PC - 1],
                scalar=corr_t[:, 0:1], in1=ed[:, 0 : PC - 1],
                op0=OP.add, op1=OP.add,
            )
        nc.vector.scalar_tensor_tensor(
            out=t2[:, PC - 1 :], in0=S[:, PC - 1 :],
            scalar=corr_t[:, 0:1], in1=ed[:, PC - 1 :],
            op0=OP.add, op1=OP.add,
        )
        nc.scalar.activation(out=ltd[:, 0:PC], in_=t2[:], func=AF.Ln)
        nc.sync.dma_start(out=out[:], in_=ltd[:])
    if legalize:
        _legalize_waits(nc, max_waits=1)
    return nc


def _pad_rows(x: np.ndarray, n: int) -> np.ndarray:
    outp = np.zeros((n,) + x.shape[1:], dtype=x.dtype)
    outp[: x.shape[0]] = x
    return outp


def kernel(greek_embeds, english_embeds, labels):
    global LAST_RESULTS, LAST_SHAPES
    g = np.ascontiguousarray(np.asarray(greek_embeds, dtype=np.float32))
    e = np.ascontiguousarray(np.asarray(english_embeds, dtype=np.float32))
    lab = np.asarray(labels)
    B, P, Hh = g.shape
    assert Hh == H and B * 2 == N_CORES

    valid = lab != IGNORE_INDEX
    pos = valid & (lab == 1)
    neg = valid & (lab != 1)
    ok = (valid.sum(-1) >= 2) & pos.any(-1) & neg.any(-1)

    count = int(pos[ok].sum()) if ok.any() else 0
    if count == 0:
        return np.float32(0.0)

    pos_idx = [np.nonzero(pos[b])[0] if ok[b] else np.zeros(0, np.int64) for b in range(B)]
    neg_idx = [np.nonzero(neg[b])[0] if ok[b] else np.zeros(0, np.int64) for b in range(B)]
    halves = [np.array_split(pi, 2) for pi in pos_idx]

    np_max = max(len(halves[b][h]) for b in range(B) for h in range(2))
    nn_max = max(len(ni) for ni in neg_idx)
    P1 = max(128, ((np_max + 127) // 128) * 128)
    N2 = max(16, ((nn_max + 15) // 16) * 16)
    N1 = max(128, ((N2 + 127) // 128) * 128)

    E15 = np.float32(np.exp(np.float32(-CMAX)))
    in_maps = []
    wvs = []
    for core in range(N_CORES):
        bb, hf = core // 2, core % 2
        p_idx = halves[bb][hf]
        n_idx = neg_idx[bb]
        w = np.zeros(P1, np.float32)
        w[: len(p_idx)] = 1.0
        wvs.append(w)
        gp_pad = _pad_rows(g[bb][p_idx].astype(ml_dtypes.bfloat16), P1)
        in_maps.append(
            {
                "gp": gp_pad,
                "gpt": np.ascontiguousarray(gp_pad.T).reshape(2, 128, P1),
                "ep": _pad_rows(e[bb][p_idx].astype(ml_dtypes.bfloat16), P1),
                "en": _pad_rows(e[bb][n_idx].astype(ml_dtypes.bfloat16), N1),
                "corr": np.array([[-(N2 - len(n_idx)) * float(E15)]], np.float32),
            }
        )

    LAST_SHAPES = (P1, N1, N2, dict(in_maps[0]))
    nc = _build_program(P1, N1, N2)
    res = run_bass_kernel_spmd(nc, in_maps, list(range(N_CORES)), trace=TRACE)
    LAST_RESULTS = res
    PC = P1 // 128
    total = 0.0
    for core, r in enumerate(res.results):
        ltd = np.asarray(r["out"], np.float64)  # [128, 2*PC]
        lt = ltd[:, 0:PC].T.reshape(-1)  # row-major per pos row
        dg = ltd[:, PC:].T.reshape(-1)
        w = wvs[core].astype(np.float64)
        total += float(np.sum(w * (lt + CMAX - dg)))
    return np.float32(total / count)
